# revision 48
# baseline (speedup 1.0000x reference)
"""Trainium2 Bass kernel for nn_BasicBlock (binarized CNN block).

Computes, data-parallel over the batch across 8 NeuronCores:
    out = hardtanh(BN1(bconv3x3(sign(x), sign(w1))) + x)
    out = hardtanh(BN2(bconv3x3(sign(out), sign(w2))) + out)
with training-mode BatchNorm whose statistics are all-reduced across
cores (exact global batch statistics, matching the reference).

Per core (8 images of the 64-image batch):
  - channels on SBUF partitions (2 groups of 128 for C=256)
  - inputs binarized to b in {1,0} (b = x>=0) stored as fp8e4 in a
    zero-padded 30x30 layout so each conv tap is a pure AP offset.
    The true sign-conv is recovered exactly via y = 2*W.b - rowsum(W)
    with rowsum(W) folded into the BN affine (all integers, exact).
  - conv = 9 taps x 2 channel-group accumulating matmuls into PSUM
    (fp8 DoubleRow); only the 28 interior columns are streamed
  - conv outputs evicted to a single shared int16 y buffer (exact)
  - x stays resident in SBUF (f32) as the layer-1 residual; B1 writes
    v = y*s + x into a second resident buffer whose clip runs in-place
    later, inside the idle stats-collective window
  - BN statistics as [sum(y), sum(y^2)] per image (Pool/Act/DVE),
    combined globally with a 2KB AllGather
  - avoids Act Sign entirely so every activation function used
    (Copy/Identity/Square/Sqrt) lives in one table: no table reloads
"""

import sys

if "/opt/trn_rl_repo" not in sys.path:
    sys.path.insert(0, "/opt/trn_rl_repo")

from contextlib import ExitStack

import numpy as np

import concourse.bass as bass
import concourse.mybir as mybir
from concourse.bass_utils import run_bass_kernel_spmd
from concourse.tile import TileContext

NCORES = 8
N_GLOBAL, C, H, W = 64, 256, 28, 28
NLOC = N_GLOBAL // NCORES  # 8 images per core
HP, WP = H + 2, W + 2      # zero-padded image
IMG, IMGP = H * W, HP * WP
CHR = 14                   # interior rows per chunk
CH = CHR * W               # 392 pixels per chunk
IMGC = 976                 # per-image padded cell: 32 margin + 900 + 44 (16-aligned)
IOFF = 32                  # image data offset inside the cell
P = 128
KG = MG = C // P           # 2 channel groups on each side
TAPS = 9
EPS = 1e-5

F32 = mybir.dt.float32
I16 = mybir.dt.int16
FP8 = mybir.dt.float8e4
AF = mybir.ActivationFunctionType
OP = mybir.AluOpType

# walrus in this container accepts at most ONE sem-wait per instruction;
# hoist extra waits onto same-engine NOPs placed just before (same queue,
# in-order dispatch -> identical semantics).
MAX_WAITS = 1
_split_ctr = [0]


def legalize_waits(nc):
    for fn in nc.m.functions:
        for bb in fn.blocks:
            out = []
            for ins in list(bb.instructions):
                si = ins.sync_info
                if si is not None and len(si.on_wait) > MAX_WAITS:
                    waits = list(si.on_wait)
                    extra, keep = waits[:-MAX_WAITS], waits[-MAX_WAITS:]
                    for w in extra:
                        _split_ctr[0] += 1
                        nop = mybir.InstNoOp(
                            name=f"I-waitsplit-{_split_ctr[0]}", engine=ins.engine
                        )
                        nop.sync_info = mybir.SyncInfo(on_wait=[w], on_update=[])
                        out.append(nop)
                    ins.sync_info = mybir.SyncInfo(
                        on_wait=keep, on_update=list(si.on_update)
                    )
                out.append(ins)
            bb.instructions = out


def build(stop_after="b2"):
    nc = bass.Bass()

    x_ext = nc.dram_tensor("x", [NLOC, C, H, W], F32, kind="ExternalInput")
    w_ext = {
        l: nc.dram_tensor(f"w{l}b", [KG, P, TAPS, MG * P], FP8, kind="ExternalInput")
        for l in (1, 2)
    }
    gm_ext = {
        l: nc.dram_tensor(f"gamma{l}", [C], F32, kind="ExternalInput") for l in (1, 2)
    }
    bt_ext = {
        l: nc.dram_tensor(f"beta{l}", [C], F32, kind="ExternalInput") for l in (1, 2)
    }
    out_ext = nc.dram_tensor("out", [NLOC, C, H, W], F32, kind="ExternalOutput")
    cc_in = {l: nc.dram_tensor(f"cc{l}_in", [MG, 2, P], F32) for l in (1, 2)}
    cc_out = {
        l: nc.dram_tensor(f"cc{l}_out", [NCORES, MG, 2, P], F32, addr_space="Shared")
        for l in (1, 2)
    }

    xv = x_ext.rearrange("n c h w -> c n (h w)")    # [256, 8, 784]
    ov = out_ext.rearrange("n c h w -> c n h w")    # [256, 8, 28, 28]

    order = ["a1", "s1", "b1a2", "s2", "b2"]
    upto = order.index(stop_after)

    with TileContext(nc) as tc:
        ctx = ExitStack()
        singles = ctx.enter_context(tc.tile_pool(name="singles", bufs=1))
        btmp = ctx.enter_context(tc.tile_pool(name="btmp", bufs=6))
        small = ctx.enter_context(tc.tile_pool(name="small", bufs=2))
        psum = ctx.enter_context(tc.tile_pool(name="psum", bufs=2, space="PSUM"))

        # ---- persistent tiles -------------------------------------------
        resx = singles.tile([P, MG, NLOC, IMG], F32, tag="resx", name="resx")
        reso = singles.tile([P, MG, NLOC, IMG], F32, tag="reso", name="reso")
        y = singles.tile([P, MG, NLOC, IMG], I16, tag="y", name="y")
        xs = {l: [singles.tile([P, KG, IMGC], FP8, tag=f"xs{l}n{n}", name=f"xs{l}n{n}")
                  for n in range(NLOC)] for l in (1, 2)}
        wsb = {l: singles.tile([P, TAPS, KG, MG * P], FP8, tag=f"wsb{l}", name=f"wsb{l}") for l in (1, 2)}
        # [sum(y), sum(y^2)] per (group, image)
        st = {l: singles.tile([P, MG, NLOC, 2], F32, tag=f"st{l}", name=f"st{l}") for l in (1, 2)}
        ttscr = singles.tile([P, IMG], F32, tag="ttscr", name="ttscr")
        plscr = singles.tile([P, IMG], F32, tag="plscr", name="plscr")
        ascr = singles.tile([P, IMG], F32, tag="ascr", name="ascr")
        gmb = {l: singles.tile([P, MG], F32, tag=f"gmb{l}", name=f"gmb{l}") for l in (1, 2)}
        btb = {l: singles.tile([P, MG], F32, tag=f"btb{l}", name=f"btb{l}") for l in (1, 2)}
        epsb = singles.tile([P, 1], F32)

        nc.vector.memset(epsb, EPS)

        # borders of the binarized tiles hold 0.5: 2*0.5-1 = 0 matches the
        # reference's zero padding of the sign values exactly
        def memset_borders(l, eng):
            for n in range(NLOC):
                t_ = xs[l][n]
                eng.memset(t_[:, :, 0:IOFF + WP], 0.5)          # margin + pad row 0
                eng.memset(t_[:, :, IMGC - 44 - WP:IMGC], 0.5)  # pad row 29 + margin
                for kg in range(KG):
                    border = bass.AP(
                        tensor=t_.tensor, offset=t_.offset + kg * IMGC + IOFF + WP,
                        ap=[list(t_.ap[0]), [WP, H], [WP - 1, 2]],
                    )
                    eng.memset(border, 0.5)

        memset_borders(1, nc.vector)

        # ---- x stream + binarize (per image), weights interleaved --------
        for n in range(NLOC):
            for mg in range(MG):
                nc.sync.dma_start(out=resx[:, mg, n, :], in_=xv[mg * P:(mg + 1) * P, n, :])
            if n == 0:
                for kg in range(KG):
                    nc.sync.dma_start(out=wsb[1][:, :, kg, :], in_=w_ext[1][kg])
            xs1v = xs[1][n][:, :, IOFF:IOFF + IMGP].rearrange("p g (r c) -> p g r c", r=HP)
            b_eng = nc.vector if n == 0 else nc.gpsimd
            b_eng.tensor_scalar(
                out=xs1v[:, :, 1:1 + H, 1:1 + W],
                in0=resx[:, :, n, :].rearrange("p g (h w) -> p g h w", h=H),
                scalar1=0.0, scalar2=None, op0=OP.is_ge,
            )

        for kg in range(KG):
            nc.sync.dma_start(out=wsb[2][:, :, kg, :], in_=w_ext[2][kg])
        for l in (1, 2):
            nc.sync.dma_start(out=gmb[l], in_=gm_ext[l].rearrange("(g p) -> p g", p=P))
            nc.sync.dma_start(out=btb[l], in_=bt_ext[l].rearrange("(g p) -> p g", p=P))

        # ---- conv for one image: 2 chunks of 14 rows, 28-col streaming ---
        def conv_image(l, n, both_act=False, both_dve=False):
            ps = {hb: psum.tile([P, MG, 512], F32, tag=f"ps{hb}", name=f"ps{hb}")
                  for hb in range(2)}
            for t in range(TAPS):
                dy, dx = t // 3 - 1, t % 3 - 1
                rhs = {}
                for hb in range(2):
                    q0 = IOFF + WP * (1 + CHR * hb + dy) + 1 + dx
                    rhs[hb] = bass.AP(
                        tensor=xs[l][n].tensor,
                        offset=xs[l][n].offset + q0,
                        ap=[list(xs[l][n].ap[0]), [IMGC, KG], [WP, CHR], [1, W]],
                    )
                for mg in range(MG):
                    lhsT = wsb[l][:, t, :, mg * P:(mg + 1) * P]
                    for hb in range(2):
                        nc.tensor.matmul(
                            ps[hb][:, mg, 0:CH], lhsT, rhs[hb],
                            start=(t == 0), stop=(t == TAPS - 1),
                            perf_mode=mybir.MatmulPerfMode.DoubleRow,
                        )
            for hb in range(2):
                if both_dve:
                    with tc.high_priority(offset=400):
                        nc.vector.tensor_scalar(out=y[:, :, n, hb * CH:(hb + 1) * CH],
                                                in0=ps[hb][:, :, 0:CH],
                                                scalar1=2.0, scalar2=None, op0=OP.mult)
                elif both_act or hb == 0:
                    nc.scalar.activation(out=y[:, :, n, hb * CH:(hb + 1) * CH],
                                         in_=ps[hb][:, :, 0:CH], func=AF.Copy,
                                         scale=2.0)
                else:
                    nc.vector.tensor_scalar(out=y[:, :, n, hb * CH:(hb + 1) * CH],
                                            in0=ps[hb][:, :, 0:CH],
                                            scalar1=2.0, scalar2=None, op0=OP.mult)

        # ---- per-image conv-output sums (emitted one image behind).
        # Last two images go on DVE so the collective launches promptly.
        def emit_stats(l, n):
            late = n >= NLOC - 2
            for mg in range(MG):
                if late:
                    with tc.high_priority(offset=400):
                        nc.vector.tensor_scalar(out=plscr, in0=y[:, mg, n, :],
                                                scalar1=0.0, scalar2=0.0,
                                                op0=OP.add, op1=OP.add,
                                                accum_out=st[l][:, mg, n, 0:1])
                    with tc.high_priority(offset=200):
                        nc.scalar.activation(out=ttscr, in_=y[:, mg, n, :],
                                             func=AF.Square,
                                             accum_out=st[l][:, mg, n, 1:2])
                else:
                    nc.vector.tensor_scalar(out=plscr, in0=y[:, mg, n, :],
                                            scalar1=0.0, scalar2=0.0,
                                            op0=OP.add, op1=OP.add,
                                            accum_out=st[l][:, mg, n, 0:1])
                    nc.scalar.activation(out=ascr, in_=y[:, mg, n, :],
                                         func=AF.Square,
                                         accum_out=st[l][:, mg, n, 1:2])

        # ---- global BN stats + affine ------------------------------------
        # y_true = 2*y_b - rowsum(W); mean_t = 2*m_b - rs, var_t = 4*var_b.
        # s = 2*gamma/sd comes free via host-side gamma doubling.
        def stats_and_affine(l):
            with tc.high_priority():
                ccsb = small.tile([P, MG, 2], F32, tag="ccsb", name="ccsb")
                nc.vector.reduce_sum(out=ccsb,
                                     in_=st[l].rearrange("p m n d -> p m d n"),
                                     axis=mybir.AxisListType.X)
                # -> [mean_b/8, E[y_b^2]/8] contribution of this core
                nc.vector.tensor_scalar(out=ccsb, in0=ccsb,
                                        scalar1=1.0 / (NLOC * IMG * NCORES),
                                        scalar2=None, op0=OP.mult)
                nc.sync.dma_start(out=cc_in[l].rearrange("g d p -> p g d"), in_=ccsb)
                nc.gpsimd.collective_compute(
                    "AllGather", OP.bypass,
                    ins=[cc_in[l][:, :, :]], outs=[cc_out[l][:, :, :, :]],
                    replica_groups=[list(range(NCORES))],
                )
                glr = small.tile([P, NCORES, MG, 2], F32, tag="glr", name="glr")
                nc.sync.dma_start(out=glr,
                                  in_=cc_out[l].rearrange("r g d p -> p r g d"))
                gl = small.tile([P, MG, 2], F32, tag="gl", name="gl")
                nc.vector.reduce_sum(out=gl, in_=glr.rearrange("p r g d -> p (g d) r"),
                                     axis=mybir.AxisListType.X)
                a, b = gl[:, :, 0:1], gl[:, :, 1:2]
                # var_t = 4*(E[y_b^2] - m_b^2) ; sd = sqrt(var_t + eps)
                var = small.tile([P, MG, 1], F32, tag="var", name="var")
                nc.vector.tensor_tensor(out=var, in0=a, in1=a, op=OP.mult)
                nc.vector.tensor_tensor(out=var, in0=b, in1=var, op=OP.subtract)
                sd = small.tile([P, MG, 1], F32, tag="sd", name="sd")
                nc.scalar.activation(out=sd, in_=var, func=AF.Sqrt, bias=epsb)
                # sT = gamma/sd
                sT = small.tile([P, MG, 1], F32, tag=f"sT{l}", name=f"sT{l}")
                nc.vector.reciprocal(out=sT, in_=sd)
                nc.vector.tensor_tensor(out=sT, in0=sT,
                                        in1=gmb[l].rearrange("p (g o) -> p g o", o=1),
                                        op=OP.mult)
                # tT = beta - m_b*sT (the 2*y_b - rowsum shift cancels in BN)
                at = small.tile([P, MG, 1], F32, tag="at", name="at")
                nc.vector.tensor_tensor(out=at, in0=a, in1=sT, op=OP.mult)
                tT = small.tile([P, MG, 1], F32, tag=f"tT{l}", name=f"tT{l}")
                nc.vector.tensor_tensor(out=tT,
                                        in0=btb[l].rearrange("p (g o) -> p g o", o=1),
                                        in1=at, op=OP.subtract)
            return {mg: (sT[:, mg, :], tT[:, mg, :]) for mg in range(MG)}

        # ---- B1: reso <- v = y*s + x; binarize (v >= -t) -> xs2; the clip
        # (add t, min 1, max -1) runs in-place in the idle collective window
        def phase_b1_image(saff, n):
            xs2v = xs[2][n][:, :, IOFF:IOFF + IMGP].rearrange("p g (r c) -> p g r c", r=HP)
            for mg in range(MG):
                s_, t_ = saff[mg]
                with tc.high_priority(offset=300):
                    nc.vector.scalar_tensor_tensor(
                        out=reso[:, mg, n, :], in0=y[:, mg, n, :], scalar=s_,
                        in1=resx[:, mg, n, :], op0=OP.mult, op1=OP.add,
                    )
                with tc.high_priority(offset=330):
                    nc.gpsimd.tensor_scalar(
                        out=xs2v[:, mg, 1:1 + H, 1:1 + W],
                        in0=reso[:, mg, n, :].rearrange("p (r c) -> p r c", c=W),
                        scalar1=t_, scalar2=0.0, op0=OP.add, op1=OP.is_ge,
                    )
                pending_mins.append((n, mg, t_))

        # ---- B2: out = clip(y*s2 + t2 + o1) -> DRAM ----------------------
        def phase_b2_image(saff, n):
            for mg in range(MG):
                s_, t_ = saff[mg]
                u0 = btmp.tile([P, IMG], F32, tag="u0")
                nc.scalar.activation(out=u0, in_=y[:, mg, n, :],
                                     func=AF.Identity, bias=t_, scale=s_)
                # o1 = max(min(v+t,1), -1) fused into the residual add
                nc.vector.scalar_tensor_tensor(
                    out=u0, in0=resx[:, mg, n, :], scalar=-1.0, in1=u0,
                    op0=OP.max, op1=OP.add)
                # reso(n) is dead after the STT above read it; write the
                # clipped output there so no staging ring is needed
                c_eng = nc.gpsimd if (n + mg) % 2 == 0 else nc.vector
                c_eng.tensor_scalar(out=reso[:, mg, n, :], in0=u0,
                                    scalar1=1.0, scalar2=-1.0,
                                    op0=OP.min, op1=OP.max)
                nc.sync.dma_start(
                    out=ov[mg * P:(mg + 1) * P, n, :, :],
                    in_=reso[:, mg, n, :].rearrange("p (r c) -> p r c", c=W),
                )

        # ---- schedule ----------------------------------------------------
        for n in range(NLOC):
            conv_image(1, n, both_dve=(n == NLOC - 1))
            if n >= 1:
                emit_stats(1, n - 1)
            if n == 2:
                # xs2 borders are only needed before B1; fill conv1 slack
                memset_borders(2, nc.gpsimd)
        emit_stats(1, NLOC - 1)
        if upto >= 1:
            saff1 = stats_and_affine(1)
        if upto >= 2:
            pending_mins = []
            phase_b1_image(saff1, 0)
            for n in range(NLOC):
                if n + 1 < NLOC:
                    phase_b1_image(saff1, n + 1)
                conv_image(2, n, both_act=(n < NLOC - 2), both_dve=(n >= NLOC - 2))
                if n >= 1:
                    emit_stats(2, n - 1)
            emit_stats(2, NLOC - 1)
        if upto >= 3:
            saff2 = stats_and_affine(2)
            # reso <- min(v+t, 1): runs in the idle collective window, before
            # B2 consumes reso (the max(-1) folds into B2's STT)
            for (n, mg, t_) in pending_mins:
                # x(n) is dead once B1's STT consumed it; reuse its storage so
                # the min is not an in-place RMW on reso
                nc.vector.tensor_scalar(out=resx[:, mg, n, :], in0=reso[:, mg, n, :],
                                        scalar1=t_, scalar2=1.0,
                                        op0=OP.add, op1=OP.min)
        if upto >= 4:
            for n in range(NLOC):
                phase_b2_image(saff2, n)
        ctx.close()

    legalize_waits(nc)
    return nc


_CACHE = {}


def kernel(x, w1, gamma1, beta1, w2, gamma2, beta2):
    if "nc" not in _CACHE:
        _CACHE["nc"] = build()
    nc = _CACHE["nc"]

    fp8np = mybir.dt.np(FP8)

    def prep_w(w):
        wb = np.where(np.asarray(w) >= 0, 1.0, -1.0).astype(np.float32)
        t = wb.reshape(MG, P, KG, P, 3, 3)       # [mg, m, kg, k, ky, kx]
        arr = t.transpose(2, 3, 4, 5, 0, 1)      # [kg, k, ky, kx, mg, m]
        return np.ascontiguousarray(arr.reshape(KG, P, TAPS, MG * P)).astype(fp8np)

    w1b, w2b = prep_w(w1), prep_w(w2)
    x = np.asarray(x, dtype=np.float32)
    g1 = np.asarray(gamma1, np.float32)
    g2 = np.asarray(gamma2, np.float32)
    b1 = np.asarray(beta1, np.float32)
    b2 = np.asarray(beta2, np.float32)

    in_maps = [
        {
            "x": np.ascontiguousarray(x[c * NLOC:(c + 1) * NLOC]),
            "w1b": w1b, "w2b": w2b,
            "gamma1": g1, "beta1": b1, "gamma2": g2, "beta2": b2,
        }
        for c in range(NCORES)
    ]
    res = run_bass_kernel_spmd(nc, in_maps, core_ids=list(range(NCORES)))
    return np.concatenate(
        [res.results[c]["out"] for c in range(NCORES)], axis=0
    ).astype(np.float32)


# revision 54
# speedup vs baseline: 1.0138x; 1.0138x over previous
"""Trainium2 Bass kernel for nn_BasicBlock (binarized CNN block).

Computes, data-parallel over the batch across 8 NeuronCores:
    out = hardtanh(BN1(bconv3x3(sign(x), sign(w1))) + x)
    out = hardtanh(BN2(bconv3x3(sign(out), sign(w2))) + out)
with training-mode BatchNorm whose statistics are all-reduced across
cores (exact global batch statistics, matching the reference).

Per core (8 images of the 64-image batch):
  - channels on SBUF partitions (2 groups of 128 for C=256)
  - inputs binarized to b in {1,0} (b = x>=0) stored as fp8e4 in a
    zero-padded 30x30 layout so each conv tap is a pure AP offset.
    The true sign-conv is recovered exactly via y = 2*W.b - rowsum(W)
    with rowsum(W) folded into the BN affine (all integers, exact).
  - conv = 9 taps x 2 channel-group accumulating matmuls into PSUM
    (fp8 DoubleRow); only the 28 interior columns are streamed
  - conv outputs evicted to a single shared int16 y buffer (exact)
  - x stays resident in SBUF (f32) as the layer-1 residual; B1 writes
    v = y*s + x into a second resident buffer whose clip runs in-place
    later, inside the idle stats-collective window
  - BN statistics as [sum(y), sum(y^2)] per image (Pool/Act/DVE),
    combined globally with a 2KB AllGather
  - avoids Act Sign entirely so every activation function used
    (Copy/Identity/Square/Sqrt) lives in one table: no table reloads
"""

import sys

if "/opt/trn_rl_repo" not in sys.path:
    sys.path.insert(0, "/opt/trn_rl_repo")

from contextlib import ExitStack

import numpy as np

import concourse.bass as bass
import concourse.mybir as mybir
from concourse.bass_utils import run_bass_kernel_spmd
from concourse.tile import TileContext

NCORES = 8
N_GLOBAL, C, H, W = 64, 256, 28, 28
NLOC = N_GLOBAL // NCORES  # 8 images per core
HP, WP = H + 2, W + 2      # zero-padded image
IMG, IMGP = H * W, HP * WP
CHR = 14                   # interior rows per chunk
CH = CHR * W               # 392 pixels per chunk
IMGC = 976                 # per-image padded cell: 32 margin + 900 + 44 (16-aligned)
IOFF = 32                  # image data offset inside the cell
P = 128
KG = MG = C // P           # 2 channel groups on each side
TAPS = 9
EPS = 1e-5

F32 = mybir.dt.float32
I16 = mybir.dt.int16
FP8 = mybir.dt.float8e4
AF = mybir.ActivationFunctionType
OP = mybir.AluOpType

# walrus in this container accepts at most ONE sem-wait per instruction;
# hoist extra waits onto same-engine NOPs placed just before (same queue,
# in-order dispatch -> identical semantics).
MAX_WAITS = 1
_split_ctr = [0]


def legalize_waits(nc):
    for fn in nc.m.functions:
        for bb in fn.blocks:
            out = []
            for ins in list(bb.instructions):
                si = ins.sync_info
                if si is not None and len(si.on_wait) > MAX_WAITS:
                    waits = list(si.on_wait)
                    extra, keep = waits[:-MAX_WAITS], waits[-MAX_WAITS:]
                    for w in extra:
                        _split_ctr[0] += 1
                        nop = mybir.InstNoOp(
                            name=f"I-waitsplit-{_split_ctr[0]}", engine=ins.engine
                        )
                        nop.sync_info = mybir.SyncInfo(on_wait=[w], on_update=[])
                        out.append(nop)
                    ins.sync_info = mybir.SyncInfo(
                        on_wait=keep, on_update=list(si.on_update)
                    )
                out.append(ins)
            bb.instructions = out


def build(stop_after="b2"):
    nc = bass.Bass()

    x_ext = nc.dram_tensor("x", [NLOC, C, H, W], F32, kind="ExternalInput")
    w_ext = {
        l: nc.dram_tensor(f"w{l}b", [KG, P, TAPS, MG * P], FP8, kind="ExternalInput")
        for l in (1, 2)
    }
    gm_ext = {
        l: nc.dram_tensor(f"gamma{l}", [C], F32, kind="ExternalInput") for l in (1, 2)
    }
    bt_ext = {
        l: nc.dram_tensor(f"beta{l}", [C], F32, kind="ExternalInput") for l in (1, 2)
    }
    out_ext = nc.dram_tensor("out", [NLOC, C, H, W], F32, kind="ExternalOutput")
    cc_in = {l: nc.dram_tensor(f"cc{l}_in", [MG, 2, P], F32) for l in (1, 2)}
    cc_out = {
        l: nc.dram_tensor(f"cc{l}_out", [NCORES, MG, 2, P], F32, addr_space="Shared")
        for l in (1, 2)
    }

    xv = x_ext.rearrange("n c h w -> c n (h w)")    # [256, 8, 784]
    ov = out_ext.rearrange("n c h w -> c n h w")    # [256, 8, 28, 28]

    order = ["a1", "s1", "b1a2", "s2", "b2"]
    upto = order.index(stop_after)

    with TileContext(nc) as tc:
        ctx = ExitStack()
        singles = ctx.enter_context(tc.tile_pool(name="singles", bufs=1))
        btmp = ctx.enter_context(tc.tile_pool(name="btmp", bufs=6))
        small = ctx.enter_context(tc.tile_pool(name="small", bufs=2))
        psum = ctx.enter_context(tc.tile_pool(name="psum", bufs=2, space="PSUM"))

        # ---- persistent tiles -------------------------------------------
        resx = singles.tile([P, MG, NLOC, IMG], F32, tag="resx", name="resx")
        reso = singles.tile([P, MG, NLOC, IMG], F32, tag="reso", name="reso")
        y = singles.tile([P, MG, NLOC, IMG], I16, tag="y", name="y")
        xs = {l: [singles.tile([P, KG, IMGC], FP8, tag=f"xs{l}n{n}", name=f"xs{l}n{n}")
                  for n in range(NLOC)] for l in (1, 2)}
        wsb = {l: singles.tile([P, TAPS, KG, MG * P], FP8, tag=f"wsb{l}", name=f"wsb{l}") for l in (1, 2)}
        # [sum(y), sum(y^2)] per (group, image)
        st = {l: singles.tile([P, MG, NLOC, 2], F32, tag=f"st{l}", name=f"st{l}") for l in (1, 2)}
        ttscr = singles.tile([P, IMG], F32, tag="ttscr", name="ttscr")
        plscr = singles.tile([P, IMG], F32, tag="plscr", name="plscr")
        ascr = singles.tile([P, IMG], F32, tag="ascr", name="ascr")
        gmb = {l: singles.tile([P, MG], F32, tag=f"gmb{l}", name=f"gmb{l}") for l in (1, 2)}
        btb = {l: singles.tile([P, MG], F32, tag=f"btb{l}", name=f"btb{l}") for l in (1, 2)}
        epsb = singles.tile([P, 1], F32)

        nc.vector.memset(epsb, EPS)

        # borders of the binarized tiles hold 0.5: 2*0.5-1 = 0 matches the
        # reference's zero padding of the sign values exactly
        def memset_borders(l, eng):
            for n in range(NLOC):
                t_ = xs[l][n]
                eng.memset(t_[:, :, 0:IOFF + WP], 0.5)          # margin + pad row 0
                eng.memset(t_[:, :, IMGC - 44 - WP:IMGC], 0.5)  # pad row 29 + margin
                for kg in range(KG):
                    border = bass.AP(
                        tensor=t_.tensor, offset=t_.offset + kg * IMGC + IOFF + WP,
                        ap=[list(t_.ap[0]), [WP, H], [WP - 1, 2]],
                    )
                    eng.memset(border, 0.5)

        memset_borders(1, nc.vector)

        # ---- x stream + binarize (per image), weights interleaved --------
        for n in range(NLOC):
            for mg in range(MG):
                nc.sync.dma_start(out=resx[:, mg, n, :], in_=xv[mg * P:(mg + 1) * P, n, :])
            if n == 0:
                for kg in range(KG):
                    nc.sync.dma_start(out=wsb[1][:, :, kg, :], in_=w_ext[1][kg])
            xs1v = xs[1][n][:, :, IOFF:IOFF + IMGP].rearrange("p g (r c) -> p g r c", r=HP)
            b_eng = nc.vector if n == 0 else nc.gpsimd
            b_eng.tensor_scalar(
                out=xs1v[:, :, 1:1 + H, 1:1 + W],
                in0=resx[:, :, n, :].rearrange("p g (h w) -> p g h w", h=H),
                scalar1=0.0, scalar2=None, op0=OP.is_ge,
            )

        for kg in range(KG):
            nc.sync.dma_start(out=wsb[2][:, :, kg, :], in_=w_ext[2][kg])
        for l in (1, 2):
            nc.sync.dma_start(out=gmb[l], in_=gm_ext[l].rearrange("(g p) -> p g", p=P))
            nc.sync.dma_start(out=btb[l], in_=bt_ext[l].rearrange("(g p) -> p g", p=P))

        # ---- conv for one image: 2 chunks of 14 rows, 28-col streaming ---
        def conv_image(l, n, both_act=False, both_dve=False):
            ps = {hb: psum.tile([P, MG, 512], F32, tag=f"ps{hb}", name=f"ps{hb}")
                  for hb in range(2)}
            for t in range(TAPS):
                dy, dx = t // 3 - 1, t % 3 - 1
                rhs = {}
                for hb in range(2):
                    q0 = IOFF + WP * (1 + CHR * hb + dy) + 1 + dx
                    rhs[hb] = bass.AP(
                        tensor=xs[l][n].tensor,
                        offset=xs[l][n].offset + q0,
                        ap=[list(xs[l][n].ap[0]), [IMGC, KG], [WP, CHR], [1, W]],
                    )
                for mg in range(MG):
                    lhsT = wsb[l][:, t, :, mg * P:(mg + 1) * P]
                    for hb in range(2):
                        nc.tensor.matmul(
                            ps[hb][:, mg, 0:CH], lhsT, rhs[hb],
                            start=(t == 0), stop=(t == TAPS - 1),
                            perf_mode=mybir.MatmulPerfMode.DoubleRow,
                        )
            for hb in range(2):
                if both_dve:
                    with tc.high_priority(offset=400):
                        nc.vector.tensor_scalar(out=y[:, :, n, hb * CH:(hb + 1) * CH],
                                                in0=ps[hb][:, :, 0:CH],
                                                scalar1=2.0, scalar2=None, op0=OP.mult)
                elif both_act or hb == 0:
                    nc.scalar.activation(out=y[:, :, n, hb * CH:(hb + 1) * CH],
                                         in_=ps[hb][:, :, 0:CH], func=AF.Copy,
                                         scale=2.0)
                else:
                    nc.vector.tensor_scalar(out=y[:, :, n, hb * CH:(hb + 1) * CH],
                                            in0=ps[hb][:, :, 0:CH],
                                            scalar1=2.0, scalar2=None, op0=OP.mult)

        # ---- per-image conv-output sums (emitted one image behind).
        # Last two images go on DVE so the collective launches promptly.
        def emit_stats(l, n):
            late = n >= NLOC - 2
            for mg in range(MG):
                if late:
                    with tc.high_priority(offset=400):
                        nc.vector.tensor_scalar(out=plscr, in0=y[:, mg, n, :],
                                                scalar1=0.0, scalar2=0.0,
                                                op0=OP.add, op1=OP.add,
                                                accum_out=st[l][:, mg, n, 0:1])
                    with tc.high_priority(offset=200):
                        nc.scalar.activation(out=ttscr, in_=y[:, mg, n, :],
                                             func=AF.Square,
                                             accum_out=st[l][:, mg, n, 1:2])
                else:
                    nc.vector.tensor_scalar(out=plscr, in0=y[:, mg, n, :],
                                            scalar1=0.0, scalar2=0.0,
                                            op0=OP.add, op1=OP.add,
                                            accum_out=st[l][:, mg, n, 0:1])
                    nc.scalar.activation(out=ascr, in_=y[:, mg, n, :],
                                         func=AF.Square,
                                         accum_out=st[l][:, mg, n, 1:2])

        # ---- global BN stats + affine ------------------------------------
        # y_true = 2*y_b - rowsum(W); mean_t = 2*m_b - rs, var_t = 4*var_b.
        # s = 2*gamma/sd comes free via host-side gamma doubling.
        def stats_and_affine(l):
            with tc.high_priority():
                ccsb = small.tile([P, MG, 2], F32, tag="ccsb", name="ccsb")
                nc.vector.reduce_sum(out=ccsb,
                                     in_=st[l].rearrange("p m n d -> p m d n"),
                                     axis=mybir.AxisListType.X)
                # -> [mean_b/8, E[y_b^2]/8] contribution of this core
                nc.vector.tensor_scalar(out=ccsb, in0=ccsb,
                                        scalar1=1.0 / (NLOC * IMG * NCORES),
                                        scalar2=None, op0=OP.mult)
                nc.sync.dma_start(out=cc_in[l].rearrange("g d p -> p g d"), in_=ccsb)
                nc.gpsimd.collective_compute(
                    "AllGather", OP.bypass,
                    ins=[cc_in[l][:, :, :]], outs=[cc_out[l][:, :, :, :]],
                    replica_groups=[list(range(NCORES))],
                )
                glr = small.tile([P, NCORES, MG, 2], F32, tag="glr", name="glr")
                nc.sync.dma_start(out=glr,
                                  in_=cc_out[l].rearrange("r g d p -> p r g d"))
                gl = small.tile([P, MG, 2], F32, tag="gl", name="gl")
                nc.vector.reduce_sum(out=gl, in_=glr.rearrange("p r g d -> p (g d) r"),
                                     axis=mybir.AxisListType.X)
                a, b = gl[:, :, 0:1], gl[:, :, 1:2]
                # var_t = 4*(E[y_b^2] - m_b^2) ; sd = sqrt(var_t + eps)
                var = small.tile([P, MG, 1], F32, tag="var", name="var")
                nc.vector.tensor_tensor(out=var, in0=a, in1=a, op=OP.mult)
                nc.vector.tensor_tensor(out=var, in0=b, in1=var, op=OP.subtract)
                sd = small.tile([P, MG, 1], F32, tag="sd", name="sd")
                nc.scalar.activation(out=sd, in_=var, func=AF.Sqrt, bias=epsb)
                # sT = gamma/sd
                sT = small.tile([P, MG, 1], F32, tag=f"sT{l}", name=f"sT{l}")
                nc.vector.reciprocal(out=sT, in_=sd)
                nc.vector.tensor_tensor(out=sT, in0=sT,
                                        in1=gmb[l].rearrange("p (g o) -> p g o", o=1),
                                        op=OP.mult)
                # tT = beta - m_b*sT (the 2*y_b - rowsum shift cancels in BN)
                at = small.tile([P, MG, 1], F32, tag="at", name="at")
                nc.vector.tensor_tensor(out=at, in0=a, in1=sT, op=OP.mult)
                tT = small.tile([P, MG, 1], F32, tag=f"tT{l}", name=f"tT{l}")
                nc.vector.tensor_tensor(out=tT,
                                        in0=btb[l].rearrange("p (g o) -> p g o", o=1),
                                        in1=at, op=OP.subtract)
            return {mg: (sT[:, mg, :], tT[:, mg, :]) for mg in range(MG)}

        # ---- B1: reso <- v = y*s + x; binarize (v >= -t) -> xs2; the clip
        # (add t, min 1, max -1) runs in-place in the idle collective window
        def phase_b1_image(saff, n):
            xs2v = xs[2][n][:, :, IOFF:IOFF + IMGP].rearrange("p g (r c) -> p g r c", r=HP)
            for mg in range(MG):
                s_, t_ = saff[mg]
                with tc.high_priority(offset=300):
                    nc.vector.scalar_tensor_tensor(
                        out=reso[:, mg, n, :], in0=y[:, mg, n, :], scalar=s_,
                        in1=resx[:, mg, n, :], op0=OP.mult, op1=OP.add,
                    )
                with tc.high_priority(offset=330):
                    nc.gpsimd.tensor_scalar(
                        out=xs2v[:, mg, 1:1 + H, 1:1 + W],
                        in0=reso[:, mg, n, :].rearrange("p (r c) -> p r c", c=W),
                        scalar1=t_, scalar2=0.0, op0=OP.add, op1=OP.is_ge,
                    )
                pending_mins.append((n, mg, t_))

        # ---- B2: out = clip(y*s2 + t2 + o1) -> DRAM ----------------------
        def phase_b2_image(saff, n):
            for mg in range(MG):
                s_, t_ = saff[mg]
                u0 = btmp.tile([P, IMG], F32, tag="u0")
                nc.scalar.activation(out=u0, in_=y[:, mg, n, :],
                                     func=AF.Identity, bias=t_, scale=s_)
                # o1 = max(min(v+t,1), -1) fused into the residual add
                nc.vector.scalar_tensor_tensor(
                    out=u0, in0=resx[:, mg, n, :], scalar=-1.0, in1=u0,
                    op0=OP.max, op1=OP.add)
                # reso(n) is dead after the STT above read it; write the
                # clipped output there so no staging ring is needed
                c_eng = nc.gpsimd if (n + mg) % 2 == 0 else nc.vector
                c_eng.tensor_scalar(out=reso[:, mg, n, :], in0=u0,
                                    scalar1=1.0, scalar2=-1.0,
                                    op0=OP.min, op1=OP.max)
                nc.sync.dma_start(
                    out=ov[mg * P:(mg + 1) * P, n, :, :],
                    in_=reso[:, mg, n, :].rearrange("p (r c) -> p r c", c=W),
                )

        # ---- schedule ----------------------------------------------------
        for n in range(NLOC):
            conv_image(1, n, both_dve=(n == NLOC - 1))
            if n >= 1:
                emit_stats(1, n - 1)
            if n == 2:
                # xs2 borders are only needed before B1; fill conv1 slack
                memset_borders(2, nc.gpsimd)
        emit_stats(1, NLOC - 1)
        if upto >= 1:
            saff1 = stats_and_affine(1)
        if upto >= 2:
            pending_mins = []
            phase_b1_image(saff1, 0)
            for n in range(NLOC):
                if n + 1 < NLOC:
                    phase_b1_image(saff1, n + 1)
                conv_image(2, n, both_act=(n < NLOC - 2), both_dve=(n >= NLOC - 2))
                if n >= 1:
                    emit_stats(2, n - 1)
            emit_stats(2, NLOC - 1)
        if upto >= 3:
            saff2 = stats_and_affine(2)
            # reso <- min(v+t, 1): runs in the idle collective window, before
            # B2 consumes reso (the max(-1) folds into B2's STT)
            for (n, mg, t_) in pending_mins:
                # x(n) is dead once B1's STT consumed it; reuse its storage so
                # the min is not an in-place RMW on reso
                nc.vector.tensor_scalar(out=resx[:, mg, n, :], in0=reso[:, mg, n, :],
                                        scalar1=t_, scalar2=1.0,
                                        op0=OP.add, op1=OP.min)
        if upto >= 4:
            for n in range(NLOC):
                phase_b2_image(saff2, n)
        ctx.close()

    legalize_waits(nc)
    return nc


_CACHE = {}


def kernel(x, w1, gamma1, beta1, w2, gamma2, beta2):
    if "nc" not in _CACHE:
        _CACHE["nc"] = build()
    nc = _CACHE["nc"]

    fp8np = mybir.dt.np(FP8)

    def prep_w(w):
        wb = np.where(np.asarray(w) >= 0, 1.0, -1.0).astype(np.float32)
        t = wb.reshape(MG, P, KG, P, 3, 3)       # [mg, m, kg, k, ky, kx]
        arr = t.transpose(2, 3, 4, 5, 0, 1)      # [kg, k, ky, kx, mg, m]
        return np.ascontiguousarray(arr.reshape(KG, P, TAPS, MG * P)).astype(fp8np)

    w1b, w2b = prep_w(w1), prep_w(w2)
    x = np.asarray(x, dtype=np.float32)
    g1 = np.asarray(gamma1, np.float32)
    g2 = np.asarray(gamma2, np.float32)
    b1 = np.asarray(beta1, np.float32)
    b2 = np.asarray(beta2, np.float32)

    in_maps = [
        {
            "x": np.ascontiguousarray(x[c * NLOC:(c + 1) * NLOC]),
            "w1b": w1b, "w2b": w2b,
            "gamma1": g1, "beta1": b1, "gamma2": g2, "beta2": b2,
        }
        for c in range(NCORES)
    ]
    res = run_bass_kernel_spmd(nc, in_maps, core_ids=list(range(NCORES)))
    return np.concatenate(
        [res.results[c]["out"] for c in range(NCORES)], axis=0
    ).astype(np.float32)


# revision 56
# speedup vs baseline: 1.1069x; 1.0918x over previous
"""Trainium2 Bass kernel for nn_BasicBlock (binarized CNN block).

Computes, data-parallel over the batch across 8 NeuronCores:
    out = hardtanh(BN1(bconv3x3(sign(x), sign(w1))) + x)
    out = hardtanh(BN2(bconv3x3(sign(out), sign(w2))) + out)
with training-mode BatchNorm whose statistics are all-reduced across
cores (exact global batch statistics, matching the reference).

Per core (8 images of the 64-image batch):
  - channels on SBUF partitions (2 groups of 128 for C=256)
  - inputs binarized to b in {1,0} (b = x>=0) stored as fp8e4 in a
    zero-padded 30x30 layout so each conv tap is a pure AP offset.
    The true sign-conv is recovered exactly via y = 2*W.b - rowsum(W)
    with rowsum(W) folded into the BN affine (all integers, exact).
  - conv = 9 taps x 2 channel-group accumulating matmuls into PSUM
    (fp8 DoubleRow); only the 28 interior columns are streamed
  - conv outputs evicted to a single shared int16 y buffer (exact)
  - x stays resident in SBUF (f32) as the layer-1 residual; B1 writes
    v = y*s + x into a second resident buffer whose clip runs in-place
    later, inside the idle stats-collective window
  - BN statistics as [sum(y), sum(y^2)] per image (Pool/Act/DVE),
    combined globally with a 2KB AllGather
  - avoids Act Sign entirely so every activation function used
    (Copy/Identity/Square/Sqrt) lives in one table: no table reloads
"""

import sys

if "/opt/trn_rl_repo" not in sys.path:
    sys.path.insert(0, "/opt/trn_rl_repo")

from contextlib import ExitStack

import numpy as np

import concourse.bass as bass
import concourse.mybir as mybir
from concourse.bass_utils import run_bass_kernel_spmd
from concourse.tile import TileContext

NCORES = 8
N_GLOBAL, C, H, W = 64, 256, 28, 28
NLOC = N_GLOBAL // NCORES  # 8 images per core
HP, WP = H + 2, W + 2      # zero-padded image
IMG, IMGP = H * W, HP * WP
CHR = 14                   # interior rows per chunk
CH = CHR * W               # 392 pixels per chunk
IMGC = 976                 # per-image padded cell: 32 margin + 900 + 44 (16-aligned)
IOFF = 32                  # image data offset inside the cell
P = 128
KG = MG = C // P           # 2 channel groups on each side
TAPS = 9
EPS = 1e-5

F32 = mybir.dt.float32
I16 = mybir.dt.int16
FP8 = mybir.dt.float8e4
AF = mybir.ActivationFunctionType
OP = mybir.AluOpType

# walrus in this container accepts at most ONE sem-wait per instruction;
# hoist extra waits onto same-engine NOPs placed just before (same queue,
# in-order dispatch -> identical semantics).
MAX_WAITS = 1
_split_ctr = [0]


def legalize_waits(nc):
    for fn in nc.m.functions:
        for bb in fn.blocks:
            out = []
            for ins in list(bb.instructions):
                si = ins.sync_info
                if si is not None and len(si.on_wait) > MAX_WAITS:
                    waits = list(si.on_wait)
                    extra, keep = waits[:-MAX_WAITS], waits[-MAX_WAITS:]
                    for w in extra:
                        _split_ctr[0] += 1
                        nop = mybir.InstNoOp(
                            name=f"I-waitsplit-{_split_ctr[0]}", engine=ins.engine
                        )
                        nop.sync_info = mybir.SyncInfo(on_wait=[w], on_update=[])
                        out.append(nop)
                    ins.sync_info = mybir.SyncInfo(
                        on_wait=keep, on_update=list(si.on_update)
                    )
                out.append(ins)
            bb.instructions = out


def build(stop_after="b2"):
    nc = bass.Bass()

    x_ext = nc.dram_tensor("x", [NLOC, C, H, W], F32, kind="ExternalInput")
    w_ext = {
        l: nc.dram_tensor(f"w{l}b", [KG, P, TAPS, MG * P], FP8, kind="ExternalInput")
        for l in (1, 2)
    }
    gm_ext = {
        l: nc.dram_tensor(f"gamma{l}", [C], F32, kind="ExternalInput") for l in (1, 2)
    }
    bt_ext = {
        l: nc.dram_tensor(f"beta{l}", [C], F32, kind="ExternalInput") for l in (1, 2)
    }
    out_ext = nc.dram_tensor("out", [NLOC, C, H, W], F32, kind="ExternalOutput")
    cc_in = {l: nc.dram_tensor(f"cc{l}_in", [MG, 2, P], F32) for l in (1, 2)}
    cc_out = {
        l: nc.dram_tensor(f"cc{l}_out", [NCORES, MG, 2, P], F32, addr_space="Shared")
        for l in (1, 2)
    }

    xv = x_ext.rearrange("n c h w -> c n (h w)")    # [256, 8, 784]
    ov = out_ext.rearrange("n c h w -> c n h w")    # [256, 8, 28, 28]

    order = ["a1", "s1", "b1a2", "s2", "b2"]
    upto = order.index(stop_after)

    with TileContext(nc) as tc:
        ctx = ExitStack()
        singles = ctx.enter_context(tc.tile_pool(name="singles", bufs=1))
        btmp = ctx.enter_context(tc.tile_pool(name="btmp", bufs=6))
        small = ctx.enter_context(tc.tile_pool(name="small", bufs=2))
        psum = ctx.enter_context(tc.tile_pool(name="psum", bufs=2, space="PSUM"))

        # ---- persistent tiles -------------------------------------------
        resx = singles.tile([P, MG, NLOC, IMG], F32, tag="resx", name="resx")
        reso = singles.tile([P, MG, NLOC, IMG], F32, tag="reso", name="reso")
        y = singles.tile([P, MG, NLOC, IMG], I16, tag="y", name="y")
        xs = {l: [singles.tile([P, KG, IMGC], FP8, tag=f"xs{l}n{n}", name=f"xs{l}n{n}")
                  for n in range(NLOC)] for l in (1, 2)}
        wsb = {l: singles.tile([P, TAPS, KG, MG * P], FP8, tag=f"wsb{l}", name=f"wsb{l}") for l in (1, 2)}
        # [sum(y), sum(y^2)] per (group, image)
        st = {l: singles.tile([P, MG, NLOC, 2], F32, tag=f"st{l}", name=f"st{l}") for l in (1, 2)}
        ttscr = singles.tile([P, IMG], F32, tag="ttscr", name="ttscr")
        plscr = singles.tile([P, IMG], F32, tag="plscr", name="plscr")
        ascr = singles.tile([P, IMG], F32, tag="ascr", name="ascr")
        gmb = {l: singles.tile([P, MG], F32, tag=f"gmb{l}", name=f"gmb{l}") for l in (1, 2)}
        btb = {l: singles.tile([P, MG], F32, tag=f"btb{l}", name=f"btb{l}") for l in (1, 2)}
        epsb = singles.tile([P, 1], F32)

        nc.vector.memset(epsb, EPS)

        # borders of the binarized tiles hold 0.5: 2*0.5-1 = 0 matches the
        # reference's zero padding of the sign values exactly
        def memset_borders(l, eng):
            for n in range(NLOC):
                t_ = xs[l][n]
                eng.memset(t_[:, :, 0:IOFF + WP], 0.5)          # margin + pad row 0
                eng.memset(t_[:, :, IMGC - 44 - WP:IMGC], 0.5)  # pad row 29 + margin
                for kg in range(KG):
                    border = bass.AP(
                        tensor=t_.tensor, offset=t_.offset + kg * IMGC + IOFF + WP,
                        ap=[list(t_.ap[0]), [WP, H], [WP - 1, 2]],
                    )
                    eng.memset(border, 0.5)

        memset_borders(1, nc.vector)

        # ---- x stream + binarize (per image), weights interleaved --------
        for n in range(NLOC):
            for mg in range(MG):
                nc.sync.dma_start(out=resx[:, mg, n, :], in_=xv[mg * P:(mg + 1) * P, n, :])
            if n == 0:
                for kg in range(KG):
                    nc.sync.dma_start(out=wsb[1][:, :, kg, :], in_=w_ext[1][kg])
            xs1v = xs[1][n][:, :, IOFF:IOFF + IMGP].rearrange("p g (r c) -> p g r c", r=HP)
            b_eng = nc.vector if n == 0 else nc.gpsimd
            b_eng.tensor_scalar(
                out=xs1v[:, :, 1:1 + H, 1:1 + W],
                in0=resx[:, :, n, :].rearrange("p g (h w) -> p g h w", h=H),
                scalar1=0.0, scalar2=None, op0=OP.is_ge,
            )

        for kg in range(KG):
            nc.sync.dma_start(out=wsb[2][:, :, kg, :], in_=w_ext[2][kg])
        for l in (1, 2):
            nc.sync.dma_start(out=gmb[l], in_=gm_ext[l].rearrange("(g p) -> p g", p=P))
            nc.sync.dma_start(out=btb[l], in_=bt_ext[l].rearrange("(g p) -> p g", p=P))

        # ---- conv for one image: 2 chunks of 14 rows, 28-col streaming ---
        def conv_image(l, n, both_act=False, both_dve=False):
            ps = {hb: psum.tile([P, MG, 512], F32, tag=f"ps{hb}", name=f"ps{hb}")
                  for hb in range(2)}
            for t in range(TAPS):
                dy, dx = t // 3 - 1, t % 3 - 1
                rhs = {}
                for hb in range(2):
                    q0 = IOFF + WP * (1 + CHR * hb + dy) + 1 + dx
                    rhs[hb] = bass.AP(
                        tensor=xs[l][n].tensor,
                        offset=xs[l][n].offset + q0,
                        ap=[list(xs[l][n].ap[0]), [IMGC, KG], [WP, CHR], [1, W]],
                    )
                for mg in range(MG):
                    lhsT = wsb[l][:, t, :, mg * P:(mg + 1) * P]
                    for hb in range(2):
                        nc.tensor.matmul(
                            ps[hb][:, mg, 0:CH], lhsT, rhs[hb],
                            start=(t == 0), stop=(t == TAPS - 1),
                            perf_mode=mybir.MatmulPerfMode.DoubleRow,
                        )
            for hb in range(2):
                if both_dve:
                    with tc.high_priority(offset=400):
                        nc.vector.tensor_scalar(out=y[:, :, n, hb * CH:(hb + 1) * CH],
                                                in0=ps[hb][:, :, 0:CH],
                                                scalar1=2.0, scalar2=None, op0=OP.mult)
                elif both_act or hb == 0:
                    nc.scalar.activation(out=y[:, :, n, hb * CH:(hb + 1) * CH],
                                         in_=ps[hb][:, :, 0:CH], func=AF.Copy,
                                         scale=2.0)
                else:
                    nc.vector.tensor_scalar(out=y[:, :, n, hb * CH:(hb + 1) * CH],
                                            in0=ps[hb][:, :, 0:CH],
                                            scalar1=2.0, scalar2=None, op0=OP.mult)

        # ---- per-image conv-output sums (emitted one image behind).
        # Last two images go on DVE so the collective launches promptly.
        def emit_stats(l, n):
            late = n >= NLOC - 2
            for mg in range(MG):
                if late:
                    with tc.high_priority(offset=400):
                        nc.vector.tensor_scalar(out=plscr, in0=y[:, mg, n, :],
                                                scalar1=0.0, scalar2=0.0,
                                                op0=OP.add, op1=OP.add,
                                                accum_out=st[l][:, mg, n, 0:1])
                    with tc.high_priority(offset=200):
                        nc.scalar.activation(out=ttscr, in_=y[:, mg, n, :],
                                             func=AF.Square,
                                             accum_out=st[l][:, mg, n, 1:2])
                else:
                    nc.vector.tensor_scalar(out=plscr, in0=y[:, mg, n, :],
                                            scalar1=0.0, scalar2=0.0,
                                            op0=OP.add, op1=OP.add,
                                            accum_out=st[l][:, mg, n, 0:1])
                    nc.scalar.activation(out=ascr, in_=y[:, mg, n, :],
                                         func=AF.Square,
                                         accum_out=st[l][:, mg, n, 1:2])

        # ---- global BN stats + affine ------------------------------------
        # y_true = 2*y_b - rowsum(W); mean_t = 2*m_b - rs, var_t = 4*var_b.
        # s = 2*gamma/sd comes free via host-side gamma doubling.
        def stats_and_affine(l):
            with tc.high_priority():
                ccsb = small.tile([P, MG, 2], F32, tag="ccsb", name="ccsb")
                nc.vector.reduce_sum(out=ccsb,
                                     in_=st[l].rearrange("p m n d -> p m d n"),
                                     axis=mybir.AxisListType.X)
                # -> [mean_b/8, E[y_b^2]/8] contribution of this core
                nc.vector.tensor_scalar(out=ccsb, in0=ccsb,
                                        scalar1=1.0 / (NLOC * IMG * NCORES),
                                        scalar2=None, op0=OP.mult)
                nc.sync.dma_start(out=cc_in[l].rearrange("g d p -> p g d"), in_=ccsb)
                nc.gpsimd.collective_compute(
                    "AllGather", OP.bypass,
                    ins=[cc_in[l][:, :, :]], outs=[cc_out[l][:, :, :, :]],
                    replica_groups=[list(range(NCORES))],
                )
                glr = small.tile([P, NCORES, MG, 2], F32, tag="glr", name="glr")
                nc.sync.dma_start(out=glr,
                                  in_=cc_out[l].rearrange("r g d p -> p r g d"))
                gl = small.tile([P, MG, 2], F32, tag="gl", name="gl")
                nc.vector.reduce_sum(out=gl, in_=glr.rearrange("p r g d -> p (g d) r"),
                                     axis=mybir.AxisListType.X)
                a, b = gl[:, :, 0:1], gl[:, :, 1:2]
                # var_t = 4*(E[y_b^2] - m_b^2) ; sd = sqrt(var_t + eps)
                var = small.tile([P, MG, 1], F32, tag="var", name="var")
                nc.vector.tensor_tensor(out=var, in0=a, in1=a, op=OP.mult)
                nc.vector.tensor_tensor(out=var, in0=b, in1=var, op=OP.subtract)
                sd = small.tile([P, MG, 1], F32, tag="sd", name="sd")
                nc.scalar.activation(out=sd, in_=var, func=AF.Sqrt, bias=epsb)
                # sT = gamma/sd
                sT = small.tile([P, MG, 1], F32, tag=f"sT{l}", name=f"sT{l}")
                nc.vector.reciprocal(out=sT, in_=sd)
                nc.vector.tensor_tensor(out=sT, in0=sT,
                                        in1=gmb[l].rearrange("p (g o) -> p g o", o=1),
                                        op=OP.mult)
                # tT = beta - m_b*sT (the 2*y_b - rowsum shift cancels in BN)
                at = small.tile([P, MG, 1], F32, tag="at", name="at")
                nc.vector.tensor_tensor(out=at, in0=a, in1=sT, op=OP.mult)
                tT = small.tile([P, MG, 1], F32, tag=f"tT{l}", name=f"tT{l}")
                nc.vector.tensor_tensor(out=tT,
                                        in0=btb[l].rearrange("p (g o) -> p g o", o=1),
                                        in1=at, op=OP.subtract)
            return {mg: (sT[:, mg, :], tT[:, mg, :]) for mg in range(MG)}

        # ---- B1: reso <- v = y*s + x; binarize (v >= -t) -> xs2; the clip
        # (add t, min 1, max -1) runs in-place in the idle collective window
        def phase_b1_image(saff, n):
            xs2v = xs[2][n][:, :, IOFF:IOFF + IMGP].rearrange("p g (r c) -> p g r c", r=HP)
            for mg in range(MG):
                s_, t_ = saff[mg]
                with tc.high_priority(offset=300):
                    nc.vector.scalar_tensor_tensor(
                        out=reso[:, mg, n, :], in0=y[:, mg, n, :], scalar=s_,
                        in1=resx[:, mg, n, :], op0=OP.mult, op1=OP.add,
                    )
                with tc.high_priority(offset=330):
                    nc.gpsimd.tensor_scalar(
                        out=xs2v[:, mg, 1:1 + H, 1:1 + W],
                        in0=reso[:, mg, n, :].rearrange("p (r c) -> p r c", c=W),
                        scalar1=t_, scalar2=0.0, op0=OP.add, op1=OP.is_ge,
                    )
                pending_mins.append((n, mg, t_))

        # ---- B2: out = clip(y*s2 + t2 + o1) -> DRAM ----------------------
        def phase_b2_image(saff, n):
            for mg in range(MG):
                s_, t_ = saff[mg]
                u0 = btmp.tile([P, IMG], F32, tag="u0")
                nc.scalar.activation(out=u0, in_=y[:, mg, n, :],
                                     func=AF.Identity, bias=t_, scale=s_)
                # o1 = max(min(v+t,1), -1) fused into the residual add
                nc.vector.scalar_tensor_tensor(
                    out=u0, in0=resx[:, mg, n, :], scalar=-1.0, in1=u0,
                    op0=OP.max, op1=OP.add)
                # reso(n) is dead after the STT above read it; write the
                # clipped output there so no staging ring is needed
                c_eng = nc.gpsimd if (n + mg) % 2 == 0 else nc.vector
                c_eng.tensor_scalar(out=reso[:, mg, n, :], in0=u0,
                                    scalar1=1.0, scalar2=-1.0,
                                    op0=OP.min, op1=OP.max)
                nc.sync.dma_start(
                    out=ov[mg * P:(mg + 1) * P, n, :, :],
                    in_=reso[:, mg, n, :].rearrange("p (r c) -> p r c", c=W),
                )

        # ---- schedule ----------------------------------------------------
        for n in range(NLOC):
            conv_image(1, n, both_dve=(n == NLOC - 1))
            if n >= 1:
                emit_stats(1, n - 1)
            if n == 2:
                # xs2 borders are only needed before B1; fill conv1 slack
                memset_borders(2, nc.gpsimd)
        emit_stats(1, NLOC - 1)
        if upto >= 1:
            saff1 = stats_and_affine(1)
        if upto >= 2:
            pending_mins = []
            phase_b1_image(saff1, 0)
            for n in range(NLOC):
                if n + 1 < NLOC:
                    phase_b1_image(saff1, n + 1)
                conv_image(2, n, both_act=(n < NLOC - 2), both_dve=(n >= NLOC - 2))
                if n >= 1:
                    emit_stats(2, n - 1)
            emit_stats(2, NLOC - 1)
        if upto >= 3:
            saff2 = stats_and_affine(2)
            # reso <- min(v+t, 1): runs in the idle collective window, before
            # B2 consumes reso (the max(-1) folds into B2's STT)
            for (n, mg, t_) in pending_mins:
                # x(n) is dead once B1's STT consumed it; reuse its storage so
                # the min is not an in-place RMW on reso
                nc.vector.tensor_scalar(out=resx[:, mg, n, :], in0=reso[:, mg, n, :],
                                        scalar1=t_, scalar2=1.0,
                                        op0=OP.add, op1=OP.min)
        if upto >= 4:
            for n in range(NLOC):
                phase_b2_image(saff2, n)
        ctx.close()

    legalize_waits(nc)
    return nc


_CACHE = {}


def kernel(x, w1, gamma1, beta1, w2, gamma2, beta2):
    if "nc" not in _CACHE:
        _CACHE["nc"] = build()
    nc = _CACHE["nc"]

    fp8np = mybir.dt.np(FP8)

    def prep_w(w):
        wb = np.where(np.asarray(w) >= 0, 1.0, -1.0).astype(np.float32)
        t = wb.reshape(MG, P, KG, P, 3, 3)       # [mg, m, kg, k, ky, kx]
        arr = t.transpose(2, 3, 4, 5, 0, 1)      # [kg, k, ky, kx, mg, m]
        return np.ascontiguousarray(arr.reshape(KG, P, TAPS, MG * P)).astype(fp8np)

    w1b, w2b = prep_w(w1), prep_w(w2)
    x = np.asarray(x, dtype=np.float32)
    g1 = np.asarray(gamma1, np.float32)
    g2 = np.asarray(gamma2, np.float32)
    b1 = np.asarray(beta1, np.float32)
    b2 = np.asarray(beta2, np.float32)

    in_maps = [
        {
            "x": np.ascontiguousarray(x[c * NLOC:(c + 1) * NLOC]),
            "w1b": w1b, "w2b": w2b,
            "gamma1": g1, "beta1": b1, "gamma2": g2, "beta2": b2,
        }
        for c in range(NCORES)
    ]
    res = run_bass_kernel_spmd(nc, in_maps, core_ids=list(range(NCORES)))
    return np.concatenate(
        [res.results[c]["out"] for c in range(NCORES)], axis=0
    ).astype(np.float32)
